# revision 6
# baseline (speedup 1.0000x reference)
"""Causal self-attention (GPT-style block) on 8 Trainium2 NeuronCores.

Problem: x[4,2048,1024] -> qkv = x@W_attn+b ; 16-head causal attention
(head_dim 64) ; out = y@W_proj+b_proj.

Sharding: tensor-parallel over heads. Core c owns heads {2c, 2c+1}:
  - computes q^T/k^T for its heads over the full batch via matmuls against
    a host-pretransposed x^T (bf16), and V in natural [key, dim] layout
    directly on the PE - no DMA transposes at all,
  - runs causal attention for its 8 (batch, head) pairs entirely in SBUF
    (S^T layout: both heads' scores in one 2-bank PSUM tile [128j, 2h, 512i];
    one fused exp per j on ScalarE; diagonal mask via a resident triangular
    bf16 mask multiplied on VectorE; PV matmul with a ones-column appended to
    V producing y_raw^T and the softmax denominator together, streaming only
    the unmasked query suffix of each j-tile),
  - per-qb normalization reads PSUM directly (denominator row copied to a
    partition-0 SBUF tile, fast reciprocal, gpsimd partition broadcast, one
    mul per head),
  - emission interleaves next batch's QKV chunks into the attention j-loops
    so the PE always has independent matmul backlog during exp waits (keeps
    the PE p-state high),
  - a fine-grained AllToAll fires after EVERY query block: core c owns rows
    {b*2048 + qb*512 + c*64 .. +64} for all (b, qb), so each qb's yT stripe
    set is a valid 8-way AllToAll of [8,128,64]. The output projection runs
    per qb-pair (128 rows) two query blocks later, fully overlapped; the
    kernel tail is just the last qb's collective + one 128-row projection.

Numerics: bf16 operands with fp32 PSUM accumulation everywhere; softmax
skips the max-subtraction (scores are O(1) by construction; exp stays
finite) which matches the reference to ~5e-3 in fp32.
"""

import numpy as np
import ml_dtypes
from contextlib import ExitStack

import concourse.bass as bass
import concourse.tile as tile
from concourse import bacc, mybir
from concourse.tile_rust import add_dep_helper
from concourse.bass_utils import run_bass_kernel_spmd

F32 = mybir.dt.float32
BF16 = mybir.dt.bfloat16
AF = mybir.ActivationFunctionType

N_CORES = 8
B, T, C, H = 4, 2048, 1024, 16
HD = C // H            # 64 head dim
HPC = H // N_CORES     # 2 heads per core
FPC = HPC * HD         # 128 features per core
BT = B * T             # 8192 rows
TCHUNK = 512           # t chunk in qkv phase
CPB = T // TCHUNK      # 4 chunks per batch
QB = 512               # query block
NQB = T // QB          # 4 per batch
KC = C // 128          # 8 contraction tiles over C
VW = 66                # [V | 1 | pad] row unit (even -> 4B-aligned offsets)
QS = QB // N_CORES     # 64-row stripe per (core, batch, qb)
ROWS = B * NQB * QS    # 1024 output rows per core, blocked (b, qb, 64)
SCALE = 1.0 / np.sqrt(HD)

LAST_RESULTS = None    # test.py reads exec_time_ns off this


def build_program(nc, debug=False):
    xT = nc.dram_tensor("xT", [C, BT], BF16, kind="ExternalInput").ap()
    wq = nc.dram_tensor("wq", [C, FPC], BF16, kind="ExternalInput").ap()
    wk = nc.dram_tensor("wk", [C, FPC], BF16, kind="ExternalInput").ap()
    wv = nc.dram_tensor("wv", [C, FPC], BF16, kind="ExternalInput").ap()
    bqkv = nc.dram_tensor("bqkv", [3, FPC], F32, kind="ExternalInput").ap()
    wp = nc.dram_tensor("wp", [C, C], BF16, kind="ExternalInput").ap()
    bp = nc.dram_tensor("bp", [C], F32, kind="ExternalInput").ap()
    out = nc.dram_tensor("out", [ROWS, C], F32, kind="ExternalOutput").ap()
    cc_in = [
        [
            nc.dram_tensor(f"cc_in{b}_{qb}", [N_CORES, FPC, QS], BF16,
                           kind="Internal").ap()
            for qb in range(NQB)
        ]
        for b in range(B)
    ]
    cc_out = [
        [
            nc.dram_tensor(f"cc_out{b}_{qb}", [N_CORES, FPC, QS], BF16,
                           kind="Internal").ap()
            for qb in range(NQB)
        ]
        for b in range(B)
    ]
    dbg = None
    if debug:
        dbg = {
            "d_qT": nc.dram_tensor("d_qT", [128, BT], BF16, kind="ExternalOutput").ap(),
            "d_kT": nc.dram_tensor("d_kT", [128, BT], BF16, kind="ExternalOutput").ap(),
            "d_vsb": nc.dram_tensor(
                "d_vsb", [128, B * (T // 128), HPC, VW], BF16, kind="ExternalOutput"
            ).ap(),
            "d_yT": nc.dram_tensor("d_yT", [128, BT], BF16, kind="ExternalOutput").ap(),
        }
    with tile.TileContext(nc) as tc:
        with ExitStack() as ctx:
            emit(ctx, tc, xT, wq, wk, wv, bqkv, wp, bp, out, cc_in, cc_out, dbg)
    return nc


def emit(ctx, tc, xT, wq, wk, wv, bqkv, wp, bp, out, cc_in, cc_out, dbg=None):
    nc = tc.nc
    res = ctx.enter_context(tc.tile_pool(name="resident", bufs=1))

    # ---------- resident SBUF ----------
    qT = res.tile([128, BT], BF16)
    kT = res.tile([128, BT], BF16)
    vsb = res.tile([128, B * (T // 128), HPC, VW], BF16)  # [V | 1 | pad] per j-tile/head
    yT = res.tile([128, BT], BF16)                        # h0 rows 0-63, h1 rows 64-127
    wq_sb = res.tile([128, KC, FPC], BF16)
    wk_sb = res.tile([128, KC, FPC], BF16)
    wv_sb = res.tile([128, KC, FPC], BF16)
    b_sb = res.tile([128, 3], F32)
    bv_sb = res.tile([128, HPC, HD], F32)
    wp_sb = res.tile([128, KC, C], BF16)
    bp_sb = res.tile([128, C], F32)
    mask3 = res.tile([128, HPC, 128], BF16)  # lower-tri (c>=p) mask, both heads

    # ---------- early constant/weight loads (wp/bp deferred) ----------
    nc.sync.dma_start(wq_sb[:], wq.rearrange("(a p) m -> p a m", p=128))
    nc.sync.dma_start(wk_sb[:], wk.rearrange("(a p) m -> p a m", p=128))
    nc.sync.dma_start(wv_sb[:], wv.rearrange("(a p) m -> p a m", p=128))
    nc.sync.dma_start(b_sb[:], bqkv.rearrange("b p -> p b"))
    bv_bcast = bass.AP(
        tensor=bqkv.tensor, offset=bqkv.offset + 2 * FPC, ap=[[0, 128], [1, FPC]]
    )
    nc.sync.dma_start(bv_sb[:], bv_bcast)
    nc.vector.memset(vsb[:, :, :, HD : HD + 1], 1.0)
    nc.vector.memset(mask3[:], 1.0)
    nc.gpsimd.affine_select(
        mask3[:], mask3[:], pattern=[[0, HPC], [1, 128]], base=0,
        channel_multiplier=-1, compare_op=mybir.AluOpType.is_ge, fill=0.0,
    )

    # ---------- pools ----------
    xpool = ctx.enter_context(tc.tile_pool(name="xt", bufs=4))
    psum = ctx.enter_context(tc.tile_pool(name="ps", bufs=3, space="PSUM"))
    ypool = ctx.enter_context(tc.tile_pool(name="yps", bufs=1, space="PSUM"))
    ptpool = ctx.enter_context(tc.tile_pool(name="pt", bufs=4))
    npool = ctx.enter_context(tc.tile_pool(name="norm", bufs=2))
    yfpool = ctx.enter_context(tc.tile_pool(name="yf", bufs=2))
    ospool = ctx.enter_context(tc.tile_pool(name="osb", bufs=2))

    xT_t = xT.rearrange("(a p) t -> p a t", p=128)
    ccs = [[None] * NQB for _ in range(B)]

    # ---------- one QKV chunk (512 tokens) of batch b ----------
    def phase1_chunk(b, ci):
        tci = CPB * b + ci
        t0 = tci * TCHUNK
        xt = xpool.tile([128, KC, TCHUNK], BF16, tag="xt")
        # split the 1 MiB chunk load across 4 DMA queues
        for spl in range(4):
            nc.sync.dma_start(
                xt[:, 2 * spl : 2 * spl + 2, :],
                xT_t[:, 2 * spl : 2 * spl + 2, t0 : t0 + TCHUNK],
            )
        ps = psum.tile([128, 2, TCHUNK], F32, tag="ps2")
        for w_sb, bi in ((wq_sb, 0), (wk_sb, 1)):
            for a in range(KC):
                nc.tensor.matmul(
                    ps[:, bi, :], lhsT=w_sb[:, a, :], rhs=xt[:, a, :],
                    start=(a == 0), stop=(a == KC - 1),
                )
        nc.vector.tensor_scalar_add(qT[:, t0 : t0 + TCHUNK], ps[:, 0, :], b_sb[:, 0:1])
        nc.vector.tensor_scalar_add(kT[:, t0 : t0 + TCHUNK], ps[:, 1, :], b_sb[:, 1:2])
        # V in natural [key, dim] layout: out[key, d] = sum_c x^T[c, key] Wv[c, d]
        vp = psum.tile([128, 4, HPC, HD], F32, tag="ps2")
        for g4 in range(4):
            for a in range(KC):
                nc.tensor.matmul(
                    vp[:, g4, :, :], lhsT=xt[:, a, g4 * 128 : g4 * 128 + 128],
                    rhs=wv_sb[:, a, :], start=(a == 0), stop=(a == KC - 1),
                )
        for g4 in range(4):
            nc.vector.tensor_add(
                vsb[:, 4 * tci + g4, :, 0:HD], vp[:, g4, :, :], bv_sb[:]
            )

    # ---------- one query block of causal attention + its AllToAll ----------
    def attention_qb(b, qb):
        q0g = b * T + qb * QB
        njt = (qb + 1) * (QB // 128)
        yps = ypool.tile([HD + 1, HPC, QB], F32, tag="yps", name=f"yp{b}_{qb}")
        for j in range(njt):
            jg = b * (T // 128) + j
            j0g = b * T + j * 128
            diag = j * 128 + 127 > qb * QB
            i0 = max(0, j * 128 - qb * QB)  # first unmasked query col
            sp = psum.tile([128, HPC, QB], F32, tag="ps2")
            for h in range(HPC):
                hs = slice(h * HD, (h + 1) * HD)
                nc.tensor.matmul(
                    sp[:, h, i0:QB], lhsT=kT[hs, j0g : j0g + 128],
                    rhs=qT[hs, q0g + i0 : q0g + QB], start=True, stop=True,
                )
            pt = ptpool.tile([128, HPC, QB], BF16, tag="pt")
            nc.scalar.activation(
                pt[:, :, i0:QB], sp[:, :, i0:QB], AF.Exp, scale=float(SCALE)
            )
            if diag:
                # boundary tile: zero above-diagonal inside the 128-wide band
                nc.vector.tensor_mul(
                    pt[:, :, i0 : i0 + 128], pt[:, :, i0 : i0 + 128], mask3[:]
                )
            for h in range(HPC):
                nc.tensor.matmul(
                    yps[:, h, i0:QB], lhsT=vsb[:, jg, h, 0 : HD + 1],
                    rhs=pt[:, h, i0:QB], start=(j == 0), stop=(j == njt - 1),
                )
        # softmax normalization: row HD of yps is the denominator. Copy it
        # to a partition-0 SBUF tile first — custom-DVE ops don't handle a
        # partition-offset PSUM read.
        ln = npool.tile([1, HPC, QB], F32, tag="ln")
        nc.vector.tensor_copy(ln[:], yps[HD : HD + 1, :, :])
        rn = npool.tile([1, HPC, QB], F32, tag="rn")
        nc.vector.reciprocal_approx_fast(rn[:], ln[:])
        rb = npool.tile([HD, HPC, QB], F32, tag="rb")
        nc.gpsimd.partition_broadcast(rb[:], rn[:], channels=HD)
        for h in range(HPC):
            nc.vector.tensor_mul(
                yT[h * HD : (h + 1) * HD, q0g : q0g + QB],
                yps[0:HD, h, :], rb[:, h, :],
            )
        # stage + fire this query block's AllToAll: one strided DMA writes
        # [r, p, t] so sender slice r is this block's 64-row stripe r.
        d = nc.sync.dma_start(
            cc_in[b][qb].rearrange("r p t -> p r t"),
            yT[:, q0g : q0g + QB].rearrange("p (r t) -> p r t", r=N_CORES),
        )
        cc = nc.gpsimd.collective_compute(
            "AllToAll", mybir.AluOpType.bypass,
            ins=[cc_in[b][qb][:]], outs=[cc_out[b][qb][:]],
            replica_groups=[list(range(N_CORES))],
        )
        add_dep_helper(cc.ins, d.ins, True, "stage before A2A")
        ccs[b][qb] = cc

    # ---------- output projection for one qb-pair (128 rows) ----------
    def proj_pair(b, pair):
        q0, q1 = 2 * pair, 2 * pair + 1
        yfull = yfpool.tile([128, KC, 2 * QS], BF16, tag="yf")
        for k, qb in enumerate((q0, q1)):
            d = nc.sync.dma_start(
                yfull[:, :, k * QS : (k + 1) * QS],
                cc_out[b][qb].rearrange("r p t -> p r t"),
            )
            add_dep_helper(d.ins, ccs[b][qb].ins, True, "gather after A2A")
        pp = psum.tile([128, 2, 512], F32, tag="ps2")
        for a in range(KC):
            lhsT = yfull[:, a, :]
            nc.tensor.matmul(pp[:, 0, :], lhsT=lhsT, rhs=wp_sb[:, a, 0:512],
                             start=(a == 0), stop=(a == KC - 1))
            nc.tensor.matmul(pp[:, 1, :], lhsT=lhsT, rhs=wp_sb[:, a, 512:C],
                             start=(a == 0), stop=(a == KC - 1))
        osb = ospool.tile([128, C], F32, tag="osb")
        nc.vector.tensor_add(osb[:, 0:512], pp[:, 0, :], bp_sb[:, 0:512])
        nc.vector.tensor_add(osb[:, 512:C], pp[:, 1, :], bp_sb[:, 512:C])
        r0 = (b * NQB + q0) * QS
        nc.sync.dma_start(out[r0 : r0 + 2 * QS, :], osb[:])

    # ---------- software pipeline over batches ----------
    # Emission order = scheduler priority. Next batch's QKV chunks are
    # interleaved into attention's query blocks so the PE has independent
    # matmul backlog whenever PV waits on exp; each qb's AllToAll fires as
    # soon as its stripes are normalized, and the matching 128-row projection
    # lands two query blocks later.
    for ci in range(CPB):
        phase1_chunk(0, ci)
    for b in range(B):
        for qb in range(NQB):
            attention_qb(b, qb)
            if b == 0 and qb == 0:
                # proj weights are first needed ~two query blocks later
                nc.sync.dma_start(wp_sb[:], wp.rearrange("(a p) e -> p a e", p=128))
                bp_bcast = bass.AP(tensor=bp.tensor, offset=bp.offset,
                                   ap=[[0, 128], [1, C]])
                nc.sync.dma_start(bp_sb[:], bp_bcast)
            if b + 1 < B:
                phase1_chunk(b + 1, qb)
            if qb == 0 and b >= 1:
                proj_pair(b - 1, 1)
            if qb == 3:
                proj_pair(b, 0)
    proj_pair(B - 1, 1)

    if dbg is not None:
        nc.sync.dma_start(dbg["d_qT"][:], qT[:])
        nc.sync.dma_start(dbg["d_kT"][:], kT[:])
        nc.sync.dma_start(dbg["d_vsb"][:], vsb[:])
        nc.sync.dma_start(dbg["d_yT"][:], yT[:])


_COMPILED_NC = None


def _get_nc():
    global _COMPILED_NC
    if _COMPILED_NC is None:
        nc = bacc.Bacc("TRN2", target_bir_lowering=False, debug=False,
                       num_devices=N_CORES)
        build_program(nc)
        nc.compile()
        _COMPILED_NC = nc
    return _COMPILED_NC


def kernel(x, W_attn, b_attn, W_proj, b_proj):
    global LAST_RESULTS
    nc = _get_nc()

    bf = ml_dtypes.bfloat16
    xT_np = np.ascontiguousarray(
        np.asarray(x, np.float32).reshape(BT, C).T
    ).astype(bf)
    W_attn = np.asarray(W_attn, np.float32)
    b_attn = np.asarray(b_attn, np.float32)
    wp_np = np.asarray(W_proj, np.float32).astype(bf)
    bp_np = np.asarray(b_proj, np.float32)

    in_maps = []
    for c in range(N_CORES):
        s = slice(c * FPC, (c + 1) * FPC)
        in_maps.append({
            "xT": xT_np,
            "wq": np.ascontiguousarray(W_attn[:, s]).astype(bf),
            "wk": np.ascontiguousarray(W_attn[:, C:2 * C][:, s]).astype(bf),
            "wv": np.ascontiguousarray(W_attn[:, 2 * C:][:, s]).astype(bf),
            "bqkv": np.ascontiguousarray(
                np.stack([b_attn[s], b_attn[C:2 * C][s], b_attn[2 * C:][s]])
            ).astype(np.float32),
            "wp": wp_np,
            "bp": bp_np,
        })

    res = run_bass_kernel_spmd(nc, in_maps, core_ids=list(range(N_CORES)))
    LAST_RESULTS = res
    # core c returns, for each (b, qb), rows {b*2048 + qb*512 + c*64 .. +64}
    full = np.empty((B, T, C), np.float32)
    fv = full.reshape(B, NQB, N_CORES, QS, C)
    for c in range(N_CORES):
        fv[:, :, c, :, :] = res.results[c]["out"].reshape(B, NQB, QS, C)
    return full


# revision 9
# speedup vs baseline: 1.2593x; 1.2593x over previous
"""Causal self-attention (GPT-style block) on 8 Trainium2 NeuronCores.

Problem: x[4,2048,1024] -> qkv = x@W_attn+b ; 16-head causal attention
(head_dim 64) ; out = y@W_proj+b_proj.

Sharding: tensor-parallel over heads. Core c owns heads {2c, 2c+1}:
  - computes q^T/k^T for its heads over the full batch via matmuls against
    a host-pretransposed x^T (bf16), and V in natural [key, dim] layout
    directly on the PE - no DMA transposes at all,
  - runs causal attention for its 8 (batch, head) pairs entirely in SBUF
    (S^T layout: both heads' scores in one 2-bank PSUM tile [128j, 2h, 512i];
    one fused exp per j on ScalarE; diagonal mask via a resident triangular
    bf16 mask multiplied on VectorE; PV matmul with a ones-column appended to
    V producing y_raw^T and the softmax denominator together, streaming only
    the unmasked query suffix of each j-tile),
  - per-qb normalization: denominator row copied to a partition-0 SBUF tile,
    fast reciprocal, gpsimd partition broadcast, one mul per head,
  - emission interleaves next batch's QKV chunks into the attention j-loops
    so the PE always has independent matmul backlog during exp waits (keeps
    the PE p-state high),
  - collectives: a dummy warm-up AllToAll fires at t~0 to absorb the
    cross-core launch-skew barrier concurrently with startup compute.
    Batches 0-2 use one striped per-batch AllToAll (core c owns rows
    {b*2048 + c*256 .. +256}); batch 3 uses four per-query-block AllToAlls
    over 64-row interleaved stripes (core c owns {3*2048 + qb*512 + c*64})
    so the kernel tail is just the last block's small collective + one
    128-row projection.

Numerics: bf16 operands with fp32 PSUM accumulation everywhere; softmax
skips the max-subtraction (scores are O(1) by construction; exp stays
finite) which matches the reference to ~5e-3 in fp32.
"""

import numpy as np
import ml_dtypes
from contextlib import ExitStack

import concourse.bass as bass
import concourse.tile as tile
from concourse import bacc, mybir
from concourse.tile_rust import add_dep_helper
from concourse.bass_utils import run_bass_kernel_spmd

F32 = mybir.dt.float32
BF16 = mybir.dt.bfloat16
AF = mybir.ActivationFunctionType

N_CORES = 8
B, T, C, H = 4, 2048, 1024, 16
HD = C // H            # 64 head dim
HPC = H // N_CORES     # 2 heads per core
FPC = HPC * HD         # 128 features per core
BT = B * T             # 8192 rows
TCHUNK = 512           # t chunk in qkv phase
CPB = T // TCHUNK      # 4 chunks per batch
QB = 512               # query block
NQB = T // QB          # 4 per batch
KC = C // 128          # 8 contraction tiles over C
VW = 66                # [V | 1 | pad] row unit (even -> 4B-aligned offsets)
STRIPE = T // N_CORES  # 256-row stripes for batches 0-2
QS = QB // N_CORES     # 64-row stripes for batch 3 (per query block)
ROWS = B * T // N_CORES  # 1024 output rows per core
SCALE = 1.0 / np.sqrt(HD)

LAST_RESULTS = None    # test.py reads exec_time_ns off this


def build_program(nc, debug=False):
    xT = nc.dram_tensor("xT", [C, BT], BF16, kind="ExternalInput").ap()
    wq = nc.dram_tensor("wq", [C, FPC], BF16, kind="ExternalInput").ap()
    wk = nc.dram_tensor("wk", [C, FPC], BF16, kind="ExternalInput").ap()
    wv = nc.dram_tensor("wv", [C, FPC], BF16, kind="ExternalInput").ap()
    bqkv = nc.dram_tensor("bqkv", [3, FPC], F32, kind="ExternalInput").ap()
    wp = nc.dram_tensor("wp", [C, C], BF16, kind="ExternalInput").ap()
    bp = nc.dram_tensor("bp", [C], F32, kind="ExternalInput").ap()
    out = nc.dram_tensor("out", [ROWS, C], F32, kind="ExternalOutput").ap()
    cc_in = [
        nc.dram_tensor(f"cc_in{b}", [N_CORES, FPC, STRIPE], BF16, kind="Internal").ap()
        for b in range(B - 1)
    ]
    cc_out = [
        nc.dram_tensor(f"cc_out{b}", [N_CORES, FPC, STRIPE], BF16, kind="Internal").ap()
        for b in range(B - 1)
    ]
    cc_in_q = [
        nc.dram_tensor(f"cc_inq{qb}", [N_CORES, FPC, QS], BF16, kind="Internal").ap()
        for qb in range(NQB)
    ]
    cc_out_q = [
        nc.dram_tensor(f"cc_outq{qb}", [N_CORES, FPC, QS], BF16, kind="Internal").ap()
        for qb in range(NQB)
    ]
    warm_in = nc.dram_tensor("warm_in", [N_CORES, 2], BF16, kind="Internal").ap()
    warm_out = nc.dram_tensor("warm_out", [N_CORES, 2], BF16, kind="Internal").ap()
    dbg = None
    if debug:
        dbg = {
            "d_qT": nc.dram_tensor("d_qT", [128, BT], BF16, kind="ExternalOutput").ap(),
            "d_kT": nc.dram_tensor("d_kT", [128, BT], BF16, kind="ExternalOutput").ap(),
            "d_vsb": nc.dram_tensor(
                "d_vsb", [128, B * (T // 128), HPC, VW], BF16, kind="ExternalOutput"
            ).ap(),
            "d_yT": nc.dram_tensor("d_yT", [128, BT], BF16, kind="ExternalOutput").ap(),
        }
    with tile.TileContext(nc) as tc:
        with ExitStack() as ctx:
            emit(ctx, tc, xT, wq, wk, wv, bqkv, wp, bp, out,
                 cc_in, cc_out, cc_in_q, cc_out_q, warm_in, warm_out, dbg)
    return nc


def emit(ctx, tc, xT, wq, wk, wv, bqkv, wp, bp, out,
         cc_in, cc_out, cc_in_q, cc_out_q, warm_in, warm_out, dbg=None):
    nc = tc.nc
    res = ctx.enter_context(tc.tile_pool(name="resident", bufs=1))

    # ---------- resident SBUF ----------
    qT = res.tile([128, BT], BF16)
    kT = res.tile([128, BT], BF16)
    vsb = res.tile([128, B * (T // 128), HPC, VW], BF16)  # [V | 1 | pad] per j-tile/head
    yT = res.tile([128, BT], BF16)                        # h0 rows 0-63, h1 rows 64-127
    wq_sb = res.tile([128, KC, FPC], BF16)
    wk_sb = res.tile([128, KC, FPC], BF16)
    wv_sb = res.tile([128, KC, FPC], BF16)
    b_sb = res.tile([128, 3], F32)
    bv_sb = res.tile([128, HPC, HD], F32)
    wp_sb = res.tile([128, KC, C], BF16)
    bp_sb = res.tile([128, C], F32)
    mask3 = res.tile([128, HPC, 128], BF16)  # lower-tri (c>=p) mask, both heads

    # ---------- warm-up collective: absorb the cross-core launch-skew
    # barrier while startup compute runs (payload values are irrelevant) ----------
    nc.gpsimd.collective_compute(
        "AllToAll", mybir.AluOpType.bypass,
        ins=[warm_in[:]], outs=[warm_out[:]],
        replica_groups=[list(range(N_CORES))],
    )

    # ---------- early constant/weight loads (wp/bp deferred) ----------
    nc.sync.dma_start(wq_sb[:], wq.rearrange("(a p) m -> p a m", p=128))
    nc.sync.dma_start(wk_sb[:], wk.rearrange("(a p) m -> p a m", p=128))
    nc.sync.dma_start(wv_sb[:], wv.rearrange("(a p) m -> p a m", p=128))
    nc.sync.dma_start(b_sb[:], bqkv.rearrange("b p -> p b"))
    bv_bcast = bass.AP(
        tensor=bqkv.tensor, offset=bqkv.offset + 2 * FPC, ap=[[0, 128], [1, FPC]]
    )
    nc.sync.dma_start(bv_sb[:], bv_bcast)
    nc.vector.memset(vsb[:, :, :, HD : HD + 1], 1.0)
    nc.vector.memset(mask3[:], 1.0)
    nc.gpsimd.affine_select(
        mask3[:], mask3[:], pattern=[[0, HPC], [1, 128]], base=0,
        channel_multiplier=-1, compare_op=mybir.AluOpType.is_ge, fill=0.0,
    )

    # ---------- pools ----------
    xpool = ctx.enter_context(tc.tile_pool(name="xt", bufs=4))
    psum = ctx.enter_context(tc.tile_pool(name="ps", bufs=3, space="PSUM"))
    ypool = ctx.enter_context(tc.tile_pool(name="yps", bufs=1, space="PSUM"))
    ptpool = ctx.enter_context(tc.tile_pool(name="pt", bufs=4))
    npool = ctx.enter_context(tc.tile_pool(name="norm", bufs=2))
    yfpool = ctx.enter_context(tc.tile_pool(name="yf", bufs=2))
    ospool = ctx.enter_context(tc.tile_pool(name="osb", bufs=2))

    xT_t = xT.rearrange("(a p) t -> p a t", p=128)
    ccs = [None] * (B - 1)
    ccqs = [None] * NQB

    # ---------- one QKV chunk (512 tokens) of batch b ----------
    def phase1_chunk(b, ci):
        tci = CPB * b + ci
        t0 = tci * TCHUNK
        xt = xpool.tile([128, KC, TCHUNK], BF16, tag="xt")
        # split the 1 MiB chunk load across 4 DMA queues
        for spl in range(4):
            nc.sync.dma_start(
                xt[:, 2 * spl : 2 * spl + 2, :],
                xT_t[:, 2 * spl : 2 * spl + 2, t0 : t0 + TCHUNK],
            )
        ps = psum.tile([128, 2, TCHUNK], F32, tag="ps2")
        for w_sb, bi in ((wq_sb, 0), (wk_sb, 1)):
            for a in range(KC):
                nc.tensor.matmul(
                    ps[:, bi, :], lhsT=w_sb[:, a, :], rhs=xt[:, a, :],
                    start=(a == 0), stop=(a == KC - 1),
                )
        nc.vector.tensor_scalar_add(qT[:, t0 : t0 + TCHUNK], ps[:, 0, :], b_sb[:, 0:1])
        nc.vector.tensor_scalar_add(kT[:, t0 : t0 + TCHUNK], ps[:, 1, :], b_sb[:, 1:2])
        # V in natural [key, dim] layout: out[key, d] = sum_c x^T[c, key] Wv[c, d]
        vp = psum.tile([128, 4, HPC, HD], F32, tag="ps2")
        for g4 in range(4):
            for a in range(KC):
                nc.tensor.matmul(
                    vp[:, g4, :, :], lhsT=xt[:, a, g4 * 128 : g4 * 128 + 128],
                    rhs=wv_sb[:, a, :], start=(a == 0), stop=(a == KC - 1),
                )
        for g4 in range(4):
            nc.vector.tensor_add(
                vsb[:, 4 * tci + g4, :, 0:HD], vp[:, g4, :, :], bv_sb[:]
            )

    # ---------- one query block of causal attention ----------
    def attention_qb(b, qb):
        q0g = b * T + qb * QB
        njt = (qb + 1) * (QB // 128)
        yps = ypool.tile([HD + 1, HPC, QB], F32, tag="yps", name=f"yp{b}_{qb}")
        for j in range(njt):
            jg = b * (T // 128) + j
            j0g = b * T + j * 128
            diag = j * 128 + 127 > qb * QB
            i0 = max(0, j * 128 - qb * QB)  # first unmasked query col
            sp = psum.tile([128, HPC, QB], F32, tag="ps2")
            for h in range(HPC):
                hs = slice(h * HD, (h + 1) * HD)
                nc.tensor.matmul(
                    sp[:, h, i0:QB], lhsT=kT[hs, j0g : j0g + 128],
                    rhs=qT[hs, q0g + i0 : q0g + QB], start=True, stop=True,
                )
            pt = ptpool.tile([128, HPC, QB], BF16, tag="pt")
            nc.scalar.activation(
                pt[:, :, i0:QB], sp[:, :, i0:QB], AF.Exp, scale=float(SCALE)
            )
            if diag:
                # boundary tile: zero above-diagonal inside the 128-wide band
                nc.vector.tensor_mul(
                    pt[:, :, i0 : i0 + 128], pt[:, :, i0 : i0 + 128], mask3[:]
                )
            for h in range(HPC):
                nc.tensor.matmul(
                    yps[:, h, i0:QB], lhsT=vsb[:, jg, h, 0 : HD + 1],
                    rhs=pt[:, h, i0:QB], start=(j == 0), stop=(j == njt - 1),
                )
        # softmax normalization: row HD of yps is the denominator. Copy it
        # to a partition-0 SBUF tile first — custom-DVE ops don't handle a
        # partition-offset PSUM read.
        ln = npool.tile([1, HPC, QB], F32, tag="ln")
        nc.vector.tensor_copy(ln[:], yps[HD : HD + 1, :, :])
        rn = npool.tile([1, HPC, QB], F32, tag="rn")
        nc.vector.reciprocal_approx_fast(rn[:], ln[:])
        rb = npool.tile([HD, HPC, QB], F32, tag="rb")
        nc.gpsimd.partition_broadcast(rb[:], rn[:], channels=HD)
        for h in range(HPC):
            nc.vector.tensor_mul(
                yT[h * HD : (h + 1) * HD, q0g : q0g + QB],
                yps[0:HD, h, :], rb[:, h, :],
            )

    # ---------- striped per-batch AllToAll (batches 0-2) ----------
    def a2a(b):
        stage = []
        for r in range(N_CORES):
            sl = slice(b * T + r * STRIPE, b * T + (r + 1) * STRIPE)
            d = nc.sync.dma_start(cc_in[b][r, :, :], yT[:, sl])
            stage.append(d)
        cc = nc.gpsimd.collective_compute(
            "AllToAll", mybir.AluOpType.bypass,
            ins=[cc_in[b][:]], outs=[cc_out[b][:]],
            replica_groups=[list(range(N_CORES))],
        )
        for d in stage:
            add_dep_helper(cc.ins, d.ins, True, "stage before A2A")
        ccs[b] = cc

    # ---------- per-query-block AllToAll (batch 3) ----------
    def a2a_qb(qb):
        q0g = (B - 1) * T + qb * QB
        d = nc.sync.dma_start(
            cc_in_q[qb].rearrange("r p t -> p r t"),
            yT[:, q0g : q0g + QB].rearrange("p (r t) -> p r t", r=N_CORES),
        )
        cc = nc.gpsimd.collective_compute(
            "AllToAll", mybir.AluOpType.bypass,
            ins=[cc_in_q[qb][:]], outs=[cc_out_q[qb][:]],
            replica_groups=[list(range(N_CORES))],
        )
        add_dep_helper(cc.ins, d.ins, True, "stage before A2A")
        ccqs[qb] = cc

    # ---------- projection helpers ----------
    def proj_tile(yfull, tt, out_r0):
        pp = psum.tile([128, 2, 512], F32, tag="ps2")
        for a in range(KC):
            lhsT = yfull[:, a, tt * 128 : (tt + 1) * 128]
            nc.tensor.matmul(pp[:, 0, :], lhsT=lhsT, rhs=wp_sb[:, a, 0:512],
                             start=(a == 0), stop=(a == KC - 1))
            nc.tensor.matmul(pp[:, 1, :], lhsT=lhsT, rhs=wp_sb[:, a, 512:C],
                             start=(a == 0), stop=(a == KC - 1))
        osb = ospool.tile([128, C], F32, tag="osb")
        nc.vector.tensor_add(osb[:, 0:512], pp[:, 0, :], bp_sb[:, 0:512])
        nc.vector.tensor_add(osb[:, 512:C], pp[:, 1, :], bp_sb[:, 512:C])
        nc.sync.dma_start(out[out_r0 : out_r0 + 128, :], osb[:])

    def proj(b):  # batches 0-2: 256 striped rows
        yfull = yfpool.tile([128, KC, STRIPE], BF16, tag="yf")
        d = nc.sync.dma_start(yfull[:], cc_out[b].rearrange("r p t -> p r t"))
        add_dep_helper(d.ins, ccs[b].ins, True, "gather after A2A")
        for tt in range(STRIPE // 128):
            proj_tile(yfull, tt, b * STRIPE + tt * 128)

    def proj_pair(pair):  # batch 3: one 128-row projection per qb-pair
        q0, q1 = 2 * pair, 2 * pair + 1
        yfull = yfpool.tile([128, KC, 2 * QS], BF16, tag="yf")
        for k, qb in enumerate((q0, q1)):
            d = nc.sync.dma_start(
                yfull[:, :, k * QS : (k + 1) * QS],
                cc_out_q[qb].rearrange("r p t -> p r t"),
            )
            add_dep_helper(d.ins, ccqs[qb].ins, True, "gather after A2A")
        proj_tile(yfull, 0, (B - 1) * STRIPE + pair * 2 * QS)

    # ---------- software pipeline ----------
    # Emission order = scheduler priority. Batch-0 QKV chunks and the next
    # batch's chunks are slid between attention query blocks so the PE has
    # independent matmul backlog during exp/DMA waits.
    phase1_chunk(0, 0)
    phase1_chunk(0, 1)
    attention_qb(0, 0)
    phase1_chunk(0, 2)
    attention_qb(0, 1)
    phase1_chunk(0, 3)
    attention_qb(0, 2)
    phase1_chunk(1, 0)
    attention_qb(0, 3)
    a2a(0)
    nc.sync.dma_start(wp_sb[:], wp.rearrange("(a p) e -> p a e", p=128))
    bp_bcast = bass.AP(tensor=bp.tensor, offset=bp.offset, ap=[[0, 128], [1, C]])
    nc.sync.dma_start(bp_sb[:], bp_bcast)
    phase1_chunk(1, 1)
    for b in (1, 2):
        attention_qb(b, 0)
        phase1_chunk(b, 2)
        attention_qb(b, 1)
        phase1_chunk(b, 3)
        proj(b - 1)
        attention_qb(b, 2)
        phase1_chunk(b + 1, 0)
        attention_qb(b, 3)
        a2a(b)
        phase1_chunk(b + 1, 1)
    # batch 3: per-query-block collectives so the tail is minimal
    attention_qb(3, 0)
    a2a_qb(0)
    phase1_chunk(3, 2)
    attention_qb(3, 1)
    a2a_qb(1)
    phase1_chunk(3, 3)
    proj(2)
    attention_qb(3, 2)
    a2a_qb(2)
    attention_qb(3, 3)
    a2a_qb(3)
    proj_pair(0)
    proj_pair(1)

    if dbg is not None:
        nc.sync.dma_start(dbg["d_qT"][:], qT[:])
        nc.sync.dma_start(dbg["d_kT"][:], kT[:])
        nc.sync.dma_start(dbg["d_vsb"][:], vsb[:])
        nc.sync.dma_start(dbg["d_yT"][:], yT[:])


_COMPILED_NC = None


def _get_nc():
    global _COMPILED_NC
    if _COMPILED_NC is None:
        nc = bacc.Bacc("TRN2", target_bir_lowering=False, debug=False,
                       num_devices=N_CORES)
        build_program(nc)
        nc.compile()
        _COMPILED_NC = nc
    return _COMPILED_NC


def kernel(x, W_attn, b_attn, W_proj, b_proj):
    global LAST_RESULTS
    nc = _get_nc()

    bf = ml_dtypes.bfloat16
    xT_np = np.ascontiguousarray(
        np.asarray(x, np.float32).reshape(BT, C).T
    ).astype(bf)
    W_attn = np.asarray(W_attn, np.float32)
    b_attn = np.asarray(b_attn, np.float32)
    wp_np = np.asarray(W_proj, np.float32).astype(bf)
    bp_np = np.asarray(b_proj, np.float32)

    in_maps = []
    for c in range(N_CORES):
        s = slice(c * FPC, (c + 1) * FPC)
        in_maps.append({
            "xT": xT_np,
            "wq": np.ascontiguousarray(W_attn[:, s]).astype(bf),
            "wk": np.ascontiguousarray(W_attn[:, C:2 * C][:, s]).astype(bf),
            "wv": np.ascontiguousarray(W_attn[:, 2 * C:][:, s]).astype(bf),
            "bqkv": np.ascontiguousarray(
                np.stack([b_attn[s], b_attn[C:2 * C][s], b_attn[2 * C:][s]])
            ).astype(np.float32),
            "wp": wp_np,
            "bp": bp_np,
        })

    res = run_bass_kernel_spmd(nc, in_maps, core_ids=list(range(N_CORES)))
    LAST_RESULTS = res
    # batches 0-2: core c owns rows {b*2048 + c*256 .. +256}
    # batch 3: core c owns rows {3*2048 + qb*512 + c*64 .. +64} per qb
    full = np.empty((B, T, C), np.float32)
    for c in range(N_CORES):
        o = res.results[c]["out"]
        for b in range(B - 1):
            full[b, c * STRIPE : (c + 1) * STRIPE, :] = o[b * STRIPE : (b + 1) * STRIPE]
        o3 = o[(B - 1) * STRIPE :].reshape(NQB, QS, C)
        full[B - 1].reshape(NQB, N_CORES, QS, C)[:, c, :, :] = o3
    return full
